# revision 17
# baseline (speedup 1.0000x reference)
"""Trainium2 Bass kernel for nn_ArmaNetwork (PyG ARMAConv x2, K=2, T=2).

kernel(**inputs) takes the FULL unsharded inputs (as produced by
reference.setup_inputs()) and returns the FULL [50000, 10] float32
log-softmax output. Internally: shards nodes across 8 NeuronCores,
builds + compiles a Bass/Tile kernel, runs it SPMD via
bass_utils.run_bass_kernel_spmd, and concatenates the per-core outputs.

Self-contained: hardcodes the problem shapes (N=50000, E=800000,
F_IN=128, HID=64, CLS=10, K=2, T=2, 8 cores).

Aggregation design (per round r=1..4, per core):
  - GCN norm factorizes: norm[e] = dinv[src]*dinv[dst]. dinv[src] is
    folded into the table rows at table-write time (scalar-engine scale
    during the transpose copy); dinv[dst] is folded into a per-round
    column scale of the accumulator. The scatter matrix S is therefore a
    pure 0/1 one-hot, built on-chip with a single is_equal against a
    persistent iota ramp. No S DMA traffic, no norm multiply.
  - edges grouped by (src-half, dest block of 128), padded per group to
    128-edge tiles uniformly across cores (SPMD structure must match).
  - r=1: pre-gathered, pre-scaled x rows streamed densely from DRAM.
  - r>=2: gpsimd.dma_gather fetches 256B table rows per edge. Descriptor
    generation runs in tile_critical(no_gpsimd_drain=True) sections so
    the Q7 engine streams desc-gen continuously; data-landing is awaited
    on the (idle) sync engine, and each chunk's first scatter matmul
    takes an explicit dep on that wait.
  - Layer-2 rounds (r=3,4) gather 20-feature tables (w2i / w2w applied
    BEFORE the table write), so the stationary operand is 20 columns.
"""

import math
from dataclasses import dataclass

import ml_dtypes
import numpy as np

import concourse.bass as bass
import concourse.mybir as mybir
import concourse.tile as tile
from concourse import bacc

BF16 = ml_dtypes.bfloat16
F32 = np.float32
DT_BF = mybir.dt.bfloat16
DT_F32 = mybir.dt.float32
DT_I16 = mybir.dt.int16
DT_U8 = mybir.dt.uint8
ALU = mybir.AluOpType
ACT = mybir.ActivationFunctionType


@dataclass
class Cfg:
    N: int = 50000
    E: int = 800000
    C: int = 8            # cores
    F_IN: int = 128
    HID: int = 64
    CLS: int = 10
    K: int = 2
    D: int = 128          # dest block width
    CHUNK_TILES: int = 16  # tiles (of 128 edges) per dma_gather call
    SPLIT: int = 3072     # per-core node split for the A/B table pipeline
    GROUP_CHUNKS: int = 6  # gather runahead depth (gpool = 2G+1)
    SINGLE_PACKET: bool = False

    @property
    def NPC(self):
        return self.N // self.C

    @property
    def NB(self):
        return math.ceil(self.NPC / self.D)

    @property
    def NT(self):
        return math.ceil(self.NPC / 128)  # node-major tile count

    @property
    def table_rows(self):
        return [self.C * self.SPLIT, self.N - self.C * self.SPLIT]


@dataclass
class Meta:
    gt: np.ndarray = None          # [2, NB] tiles per (half, block)
    tile_block: list = None        # per half: np int array tile -> block
    block_first: list = None       # per half: block -> first tile idx (in half)
    block_last: list = None        # per half: block -> last tile idx
    half_tiles: list = None        # [TH0, TH1]
    chunks: list = None            # per half: list of (t0, ntiles)
    TT: int = 0


def np_ref(x, edge_index, w, cfg: Cfg):
    """Numpy mirror of reference() for arbitrary sizes."""
    N = x.shape[0]
    row, col = edge_index[0].astype(np.int64), edge_index[1].astype(np.int64)
    deg = np.bincount(col, minlength=N).astype(F32)
    dinv = np.where(deg > 0, 1.0 / np.sqrt(deg), 0.0).astype(F32)
    norm = dinv[row] * dinv[col]

    def agg(v):
        out = np.zeros_like(v)
        np.add.at(out, col, v[row] * norm[:, None])
        return out

    def arma(xx, init_w, ww, root_w, bias, T):
        root = np.einsum('nf,kfo->nko', xx, root_w[0]) + bias[0][:, 0, :]
        out = np.einsum('nf,kfo->nko', xx, init_w)
        for t in range(T):
            if t > 0:
                out = np.einsum('nko,koh->nkh', out, ww[0])
            K = out.shape[1]
            out = np.stack([agg(out[:, k]) for k in range(K)], 1)
            out = out + root
            if t < T - 1:
                out = np.maximum(out, 0)
        return out.mean(axis=1)

    h = np.maximum(arma(x, w['c1_init'], w['c1_w'], w['c1_root'], w['c1_bias'], 2), 0)
    h = np.maximum(arma(h, w['c2_init'], w['c2_w'], w['c2_root'], w['c2_bias'], 2), 0)
    m = h.max(axis=1, keepdims=True)
    ls = h - m - np.log(np.exp(h - m).sum(axis=1, keepdims=True))
    return ls.astype(F32)


def host_prep(x, edge_index, w, cfg: Cfg):
    """Returns (in_maps, meta)."""
    N, E, C, D = cfg.N, cfg.E, cfg.C, cfg.D
    NPC, NB, SPLIT = cfg.NPC, cfg.NB, cfg.SPLIT
    row = edge_index[0].astype(np.int64)
    col = edge_index[1].astype(np.int64)
    deg = np.bincount(col, minlength=N).astype(F32)
    dinv = np.where(deg > 0, 1.0 / np.sqrt(deg), 0.0).astype(F32)

    core_of = col // NPC
    per_core = []
    counts = np.zeros((C, 2, NB), dtype=np.int64)
    for c in range(C):
        sel = np.nonzero(core_of == c)[0]
        r = row[sel]
        d = col[sel] - c * NPC
        h = ((r % NPC) >= SPLIT).astype(np.int64)
        b = d // D
        order = np.lexsort((r, b, h))
        r, d, h, b = r[order], d[order], h[order], b[order]
        for hh in (0, 1):
            counts[c, hh] = np.bincount(b[h == hh], minlength=NB)
        per_core.append((r, d, h, b))

    gt = np.maximum(1, np.ceil(counts.max(axis=0) / 128).astype(np.int64))  # [2, NB]
    half_tiles = [int(gt[0].sum()), int(gt[1].sum())]
    TT = half_tiles[0] + half_tiles[1]
    TTe = TT * 128

    tile_block, block_first, block_last = [], [], []
    for hh in (0, 1):
        tb = np.repeat(np.arange(NB), gt[hh])
        tile_block.append(tb)
        first = np.zeros(NB, dtype=np.int64)
        last = np.zeros(NB, dtype=np.int64)
        pos = 0
        for b in range(NB):
            first[b] = pos
            pos += gt[hh][b]
            last[b] = pos - 1
        block_first.append(first)
        block_last.append(last)

    chunks = []
    for hh in (0, 1):
        ch = []
        t0 = 0
        while t0 < half_tiles[hh]:
            nt = min(cfg.CHUNK_TILES, half_tiles[hh] - t0)
            ch.append((t0, nt))
            t0 += nt
        chunks.append(ch)

    meta = Meta(gt=gt, tile_block=tile_block, block_first=block_first,
                block_last=block_last, half_tiles=half_tiles, chunks=chunks, TT=TT)

    # slot base offset (in tiles) for (half, block)
    slot_base = np.zeros((2, NB), dtype=np.int64)
    for hh in (0, 1):
        base = 0 if hh == 0 else half_tiles[0]
        slot_base[hh] = base + np.concatenate(([0], np.cumsum(gt[hh])[:-1]))

    x_scaled_bf = (x * dinv[:, None]).astype(BF16)   # dinv[src]-folded x
    # weights packing (shared across cores)
    K, HID, CLS = cfg.K, cfg.HID, cfg.CLS
    w1i = np.zeros((128, 128), F32)
    w1r = np.zeros((128, 128), F32)
    b1 = np.zeros((128, 1), F32)
    w1w = np.zeros((128, 128), F32)
    m1 = np.zeros((128, HID), F32)
    for k in range(K):
        w1i[:, k * HID:(k + 1) * HID] = w['c1_init'][k]
        w1r[:, k * HID:(k + 1) * HID] = w['c1_root'][0][k]
        b1[k * HID:(k + 1) * HID, 0] = w['c1_bias'][0][k, 0]
        w1w[k * HID:(k + 1) * HID, k * HID:(k + 1) * HID] = w['c1_w'][0][k]
        m1[k * HID:(k + 1) * HID, :] = np.eye(HID, dtype=F32) / K
    KC = K * CLS
    w2i = np.zeros((128, KC), F32)
    w2r = np.zeros((128, KC), F32)
    b2 = np.zeros((KC, 1), F32)
    w2w = np.zeros((KC, KC), F32)
    m2 = np.zeros((KC, CLS), F32)
    for k in range(K):
        w2i[:HID, k * CLS:(k + 1) * CLS] = w['c2_init'][k]
        w2r[:HID, k * CLS:(k + 1) * CLS] = w['c2_root'][0][k]
        b2[k * CLS:(k + 1) * CLS, 0] = w['c2_bias'][0][k, 0]
        w2w[k * CLS:(k + 1) * CLS, k * CLS:(k + 1) * CLS] = w['c2_w'][0][k]
        m2[k * CLS:(k + 1) * CLS, :] = np.eye(CLS, dtype=F32) / K
    shared = {
        'w1i': w1i.astype(BF16), 'w1r': w1r.astype(BF16), 'b1': b1,
        'w1w': w1w.astype(BF16), 'm1': m1.astype(BF16),
        'w2i': w2i.astype(BF16), 'w2r': w2r.astype(BF16), 'b2': b2,
        'w2w': w2w.astype(BF16), 'm2': m2.astype(BF16),
        'idb': np.eye(128, dtype=F32).astype(BF16),
    }

    in_maps = []
    for c in range(C):
        r, d, h, b = per_core[c]
        src = np.zeros(TTe, dtype=np.int64)
        dloc = np.full(TTe, 200.0, dtype=F32)   # pad -> never matches iota
        # place each (h, b) group at its slot range
        pos_in_group = np.zeros(len(r), dtype=np.int64)
        key = h * NB + b
        order_stable = np.argsort(key, kind='stable')
        ks = key[order_stable]
        cc = np.arange(len(ks)) - np.concatenate(
            ([0], np.cumsum(np.bincount(ks, minlength=2 * NB))[:-1]))[ks]
        pos_in_group[order_stable] = cc
        slots = slot_base[h, b] * 128 + pos_in_group
        src[slots] = r
        dloc[slots] = (d % D).astype(F32)
        # pad slots keep src=0 -> idx 0 of either table; dloc=200 makes
        # their S rows all-zero, so the fetched data is ignored.
        sc = src // NPC
        so = src % NPC
        memb = (so >= SPLIT).astype(np.int64)
        idxm = np.where(memb == 0, sc * SPLIT + so,
                        sc * (NPC - SPLIT) + (so - SPLIT))
        pad_mask = np.ones(TTe, dtype=bool)
        pad_mask[slots] = False
        idxm[pad_mask] = 0

        idx16 = idxm.astype(np.int16)
        idx_wr = np.tile(np.ascontiguousarray(idx16.reshape(-1, 16).T), (8, 1))

        # per-slot dloc in [128, TT, 1] (slot e -> partition e%128, tile e//128)
        dloc_sw = np.ascontiguousarray(
            dloc.reshape(TT, 128).T.reshape(128, TT, 1)).astype(np.uint8)

        g1 = x_scaled_bf[src]  # [TTe, 128], already dinv[src]-scaled
        g1_sw = np.ascontiguousarray(
            g1.reshape(TT, 128, cfg.F_IN).transpose(1, 0, 2))

        xT = np.ascontiguousarray(
            x.astype(BF16)[c * NPC:(c + 1) * NPC].T)  # [128, NPC] unscaled

        dl = dinv[c * NPC:(c + 1) * NPC]
        NTc = math.ceil(NPC / 128)
        dpad = np.zeros(NTc * 128, F32)
        dpad[:NPC] = dl
        dinvn = np.ascontiguousarray(dpad.reshape(NTc, 128).T)  # [128, NT]
        dinvb = np.ascontiguousarray(
            np.broadcast_to(dl.astype(BF16), (128, NPC)))       # [128, NPC]

        im = {'g1': g1_sw, 'dloc': dloc_sw, 'idx': idx_wr, 'xT': xT,
              'dinvn': dinvn, 'dinvb': dinvb}
        im.update(shared)
        in_maps.append(im)

    return in_maps, meta


def _dep(a, b, sync):
    ia = a.ins if hasattr(a, 'ins') else a
    ib = b.ins if hasattr(b, 'ins') else b
    bass._add_dep_helper(ia, ib, sync=sync, reason="arma-dep")


class Builder:
    def __init__(self, cfg: Cfg, meta: Meta):
        self.cfg = cfg
        self.meta = meta
        self.qrr = 0
        self.qcnt = [0, 0, 0, 0]
        self.last_issue = [None, None, None, None]

    def build(self):
        cfg, meta = self.cfg, self.meta
        NPC, NB, NT, TT = cfg.NPC, cfg.NB, cfg.NT, meta.TT
        KC = cfg.K * cfg.CLS
        nc = bacc.Bacc(None, target_bir_lowering=False,
                       num_swdge_queues=4, num_devices=cfg.C,
                       detect_race_conditions=False)
        self.nc = nc
        g1 = nc.dram_tensor("g1", [128, TT, 128], DT_BF, kind="ExternalInput")
        dlocd = nc.dram_tensor("dloc", [128, TT, 1], DT_U8, kind="ExternalInput")
        idxw = nc.dram_tensor("idx", [128, TT * 8], DT_I16, kind="ExternalInput")
        xTd = nc.dram_tensor("xT", [128, NPC], DT_BF, kind="ExternalInput")
        dinvnd = nc.dram_tensor("dinvn", [128, NT], DT_F32, kind="ExternalInput")
        dinvbd = nc.dram_tensor("dinvb", [128, NPC], DT_BF, kind="ExternalInput")
        w1i = nc.dram_tensor("w1i", [128, 128], DT_BF, kind="ExternalInput")
        w1r = nc.dram_tensor("w1r", [128, 128], DT_BF, kind="ExternalInput")
        b1 = nc.dram_tensor("b1", [128, 1], DT_F32, kind="ExternalInput")
        w1w = nc.dram_tensor("w1w", [128, 128], DT_BF, kind="ExternalInput")
        m1 = nc.dram_tensor("m1", [128, cfg.HID], DT_BF, kind="ExternalInput")
        w2i = nc.dram_tensor("w2i", [128, KC], DT_BF, kind="ExternalInput")
        w2r = nc.dram_tensor("w2r", [128, KC], DT_BF, kind="ExternalInput")
        b2 = nc.dram_tensor("b2", [KC, 1], DT_F32, kind="ExternalInput")
        w2w = nc.dram_tensor("w2w", [KC, KC], DT_BF, kind="ExternalInput")
        m2 = nc.dram_tensor("m2", [KC, cfg.CLS], DT_BF, kind="ExternalInput")
        idb = nc.dram_tensor("idb", [128, 128], DT_BF, kind="ExternalInput")
        outd = nc.dram_tensor("out", [NPC, cfg.CLS], DT_F32, kind="ExternalOutput")

        from contextlib import ExitStack
        with tile.TileContext(nc) as tc, ExitStack() as stk:
            self.tc = tc
            pers = stk.enter_context(tc.tile_pool(name="pers", bufs=1))
            dramp = stk.enter_context(
                tc.tile_pool(name="dramp", bufs=1, space="DRAM"))
            gpool = stk.enter_context(
                tc.tile_pool(name="gpool", bufs=2 * cfg.GROUP_CHUNKS))
            spool = stk.enter_context(tc.tile_pool(name="spool", bufs=5))
            pspool = stk.enter_context(
                tc.tile_pool(name="pspool", bufs=4, space="PSUM"))
            psd = stk.enter_context(
                tc.tile_pool(name="psd", bufs=2, space="PSUM"))
            pst = stk.enter_context(
                tc.tile_pool(name="pst", bufs=2, space="PSUM"))
            self.gpool, self.spool, self.pspool = gpool, spool, pspool
            self.psd, self.pst = psd, pst

            idx_t = pers.tile([128, TT * 8], DT_I16, name="idx_t", tag="idx_t")
            nc.sync.dma_start(idx_t[:], idxw[:])
            dloc_t = pers.tile([128, TT, 1], DT_U8, name="dloc_t", tag="dloc_t")
            nc.sync.dma_start(dloc_t[:], dlocd[:])
            dinvn_t = pers.tile([128, NT], DT_F32, name="dinvn_t", tag="dinvn_t")
            nc.sync.dma_start(dinvn_t[:], dinvnd[:])
            dinvb_t = pers.tile([128, NPC], DT_BF, name="dinvb_t", tag="dinvb_t")
            nc.sync.dma_start(dinvb_t[:], dinvbd[:])
            wt = {}
            for nm_, dr, shape, dt in [
                ("w1i", w1i, [128, 128], DT_BF), ("w1r", w1r, [128, 128], DT_BF),
                ("b1", b1, [128, 1], DT_F32), ("w1w", w1w, [128, 128], DT_BF),
                ("m1", m1, [128, cfg.HID], DT_BF), ("w2i", w2i, [128, KC], DT_BF),
                ("w2r", w2r, [128, KC], DT_BF), ("b2", b2, [KC, 1], DT_F32),
                ("w2w", w2w, [KC, KC], DT_BF), ("m2", m2, [KC, cfg.CLS], DT_BF),
                ("idb", idb, [128, 128], DT_BF),
            ]:
                t = pers.tile(shape, dt, name=f"{nm_}_t", tag=f"{nm_}_t")
                nc.sync.dma_start(t[:], dr[:])
                wt[nm_] = t
            self.wt = wt
            self.dinvn_t, self.dinvb_t = dinvn_t, dinvb_t

            # iota ramp 0..127 along the innermost dim, replicated over
            # CHUNK_TILES; bf16 is exact for 0..127.
            iota_t = pers.tile([128, cfg.CHUNK_TILES, 128], DT_U8,
                               name="iota_t", tag="iota_t")
            nc.gpsimd.iota(iota_t[:], pattern=[[0, cfg.CHUNK_TILES], [1, 128]],
                           base=0, channel_multiplier=0,
                           allow_small_or_imprecise_dtypes=True)
            self.iota_t = iota_t
            self.dloc_t = dloc_t

            acc = pers.tile([128, NPC], DT_BF, name="acc", tag="acc")
            mT = pers.tile([128, NPC], DT_BF, name="mT", tag="mT")
            root1 = pers.tile([128, NPC], DT_BF, name="root1", tag="root1")
            hT = pers.tile([128, NPC], DT_BF, name="hT", tag="hT")
            root2 = pers.tile([KC, NPC], DT_BF, name="root2", tag="root2")
            t20 = pers.tile([KC, NPC], DT_BF, name="t20", tag="t20")
            zT32 = pers.tile([32, NPC], DT_BF, name="zT32", tag="zT32")
            nm = pers.tile([128, NT, 128], DT_BF, name="nm", tag="nm")
            self.acc = acc

            tin = [dramp.tile([NPC, 128], DT_BF, name=f"tin{r}", tag=f"tin{r}")
                   for r in range(3)]
            rowsA, rowsB = cfg.table_rows
            tfullA = [dramp.tile([rowsA, 128], DT_BF, addr_space="Shared",
                                 name=f"tfullA{r}", tag=f"tfullA{r}")
                      for r in range(3)]
            tfullB = [dramp.tile([rowsB, 128], DT_BF, addr_space="Shared",
                                 name=f"tfullB{r}", tag=f"tfullB{r}")
                      for r in range(3)]

            def coll_A(r):
                nc.gpsimd.collective_compute(
                    "AllGather", ALU.bypass,
                    replica_groups=[list(range(cfg.C))],
                    ins=[tin[r][0:cfg.SPLIT, :]], outs=[tfullA[r][:]])

            def coll_B(r):
                nc.gpsimd.collective_compute(
                    "AllGather", ALU.bypass,
                    replica_groups=[list(range(cfg.C))],
                    ins=[tin[r][cfg.SPLIT:NPC, :]], outs=[tfullB[r][:]])
            self.g1, self.idx_t = g1, idx_t
            self.qsem = [nc.alloc_semaphore(name=f"qsem{i}") for i in range(4)]

            node_chunks = []
            c0 = 0
            while c0 < NPC:
                w_ = min(512, NPC - c0)
                node_chunks.append((c0, w_))
                c0 += 512
            self.node_chunks = node_chunks

            # ---- R1: root1 = x @ w1r + b1 (xT streamed per chunk) ----
            xpool = stk.enter_context(tc.tile_pool(name="xpool", bufs=2))
            for (c0, w_) in node_chunks:
                xc = xpool.tile([128, 512], DT_BF, name="xc", tag="xc")
                nc.sync.dma_start(xc[:, :w_], xTd[:, c0:c0 + w_])
                ps = psd.tile([128, 512], DT_F32, name="psd", tag="psd")
                nc.tensor.matmul(ps[:, :w_], wt["w1r"][:], xc[:, :w_],
                                 start=True, stop=True)
                nc.scalar.activation(root1[:, c0:c0 + w_], ps[:, :w_],
                                     ACT.Identity, bias=wt["b1"][:])

            def table_chunk(srcT, rows, tin, c0, w_):
                # transpose srcT[:rows, c0:c0+w_] into node-major table rows,
                # scaling each node row by dinv[node]; one DMA per chunk.
                t0 = c0 // 128
                t1 = (c0 + w_ + 127) // 128
                for t in range(t0, t1):
                    tw = min(128, NPC - t * 128)
                    pt = self.pst.tile([128, 128], DT_BF, name="pst", tag="pst")
                    nc.tensor.transpose(pt[:tw, :rows],
                                        srcT[:rows, t * 128:t * 128 + tw],
                                        wt["idb"][:rows, :rows])
                    nc.scalar.activation(nm[:tw, t, :rows], pt[:tw, :rows],
                                         ACT.Identity,
                                         scale=self.dinvn_t[:tw, t:t + 1])
                nfull_c = min(t1 * 128, NPC // 128 * 128)
                if nfull_c > c0:
                    nc.sync.dma_start(
                        tin[c0:nfull_c, :].rearrange("(t p) f -> p t f", p=128),
                        nm[:, t0:nfull_c // 128, :])
                if t1 * 128 > NPC:
                    tailw = NPC - NPC // 128 * 128
                    nc.sync.dma_start(tin[NPC - tailw:NPC, :],
                                      nm[:tailw, NPC // 128, :])

            # ---- R1 aggregation: acc = (A @ x)^T; per-chunk: out1 -> mT ----
            def post1(k, c0, w_):
                ps = psd.tile([128, 512], DT_F32, name="psd", tag="psd")
                nc.tensor.matmul(ps[:, :w_], wt["w1i"][:], acc[:, c0:c0 + w_],
                                 start=True, stop=True)
                nc.vector.tensor_tensor(mT[:, c0:c0 + w_], ps[:, :w_],
                                        root1[:, c0:c0 + w_], op=ALU.add)
                nc.vector.tensor_scalar_max(mT[:, c0:c0 + w_],
                                            mT[:, c0:c0 + w_], 0.0)
                table_chunk(mT, 128, tin[0], c0, w_)

            self.aggregate(1, None, None, post_cb=post1,
                           mid_cb=lambda: coll_A(0))
            coll_B(0)

            # ---- R2 ----
            nc.vector.memset(hT[cfg.HID:, :], 0.0)
            nc.vector.memset(zT32[:, :], 0.0)

            def post2(k, c0, w_):
                ps = psd.tile([128, 512], DT_F32, name="psd", tag="psd")
                nc.tensor.matmul(ps[:, :w_], wt["w1w"][:], acc[:, c0:c0 + w_],
                                 start=True, stop=True)
                nc.vector.tensor_tensor(mT[:, c0:c0 + w_], ps[:, :w_],
                                        root1[:, c0:c0 + w_], op=ALU.add)
                ps2 = psd.tile([128, 512], DT_F32, name="psd", tag="psd")
                nc.tensor.matmul(ps2[:cfg.HID, :w_], wt["m1"][:],
                                 mT[:, c0:c0 + w_], start=True, stop=True)
                nc.scalar.activation(hT[:cfg.HID, c0:c0 + w_],
                                     ps2[:cfg.HID, :w_], ACT.Relu)
                ps3 = psd.tile([128, 512], DT_F32, name="psd", tag="psd")
                nc.tensor.matmul(ps3[:KC, :w_], wt["w2r"][:], hT[:, c0:c0 + w_],
                                 start=True, stop=True)
                nc.scalar.activation(root2[:, c0:c0 + w_], ps3[:KC, :w_],
                                     ACT.Identity, bias=wt["b2"][:])
                ps4 = psd.tile([128, 512], DT_F32, name="psd", tag="psd")
                nc.tensor.matmul(ps4[:KC, :w_], wt["w2i"][:], hT[:, c0:c0 + w_],
                                 start=True, stop=True)
                nc.vector.tensor_copy(zT32[:KC, c0:c0 + w_], ps4[:KC, :w_])
                table_chunk(zT32, 32, tin[1], c0, w_)

            self.aggregate(2, tfullA[0], tfullB[0], post_cb=post2,
                           mid_cb=lambda: coll_A(1))
            coll_B(1)

            # ---- R3 ----
            def post3(k, c0, w_):
                nc.vector.tensor_tensor(hT[:KC, c0:c0 + w_],
                                        acc[:KC, c0:c0 + w_],
                                        root2[:, c0:c0 + w_], op=ALU.add)
                nc.vector.tensor_scalar_max(hT[:KC, c0:c0 + w_],
                                            hT[:KC, c0:c0 + w_], 0.0)
                ps = psd.tile([128, 512], DT_F32, name="psd", tag="psd")
                nc.tensor.matmul(ps[:KC, :w_], wt["w2w"][:],
                                 hT[:KC, c0:c0 + w_], start=True, stop=True)
                nc.vector.tensor_copy(zT32[:KC, c0:c0 + w_], ps[:KC, :w_])
                table_chunk(zT32, 32, tin[2], c0, w_)

            self.aggregate(3, tfullA[1], tfullB[1], fcols=KC, post_cb=post3,
                           mid_cb=lambda: coll_A(2))
            coll_B(2)

            # ---- R4 ----
            ys = pers.tile([128, NT, cfg.CLS], DT_F32, name="ys", tag="ys")

            rmax = pers.tile([128, NT, 1], DT_F32, name="rmax", tag="rmax")
            xm = pers.tile([128, NT, cfg.CLS], DT_F32, name="xm", tag="xm")
            ex = pers.tile([128, NT, cfg.CLS], DT_F32, name="ex", tag="ex")
            ssum = pers.tile([128, NT, 1], DT_F32, name="ssum", tag="ssum")
            lgs = pers.tile([128, NT, 1], DT_F32, name="lgs", tag="lgs")
            res = xm

            def post4(k, c0, w_):
                nc.vector.tensor_tensor(t20[:, c0:c0 + w_],
                                        acc[:KC, c0:c0 + w_],
                                        root2[:, c0:c0 + w_], op=ALU.add)
                ps = psd.tile([128, 512], DT_F32, name="psd", tag="psd")
                nc.tensor.matmul(ps[:cfg.CLS, :w_], wt["m2"][:],
                                 t20[:, c0:c0 + w_], start=True, stop=True)
                nc.scalar.activation(mT[:cfg.CLS, c0:c0 + w_],
                                     ps[:cfg.CLS, :w_], ACT.Relu)
                t0 = c0 // 128
                t1 = (c0 + w_ + 127) // 128
                for t in range(t0, t1):
                    tw = min(128, NPC - t * 128)
                    pt = self.pst.tile([128, 128], DT_BF, name="pst", tag="pst")
                    nc.tensor.transpose(pt[:tw, :], mT[:, t * 128:t * 128 + tw],
                                        wt["idb"][:])
                    nc.vector.tensor_copy(ys[:tw, t, :], pt[:tw, :cfg.CLS])
                # log_softmax + output for this chunk's node tiles
                sl = (slice(None), slice(t0, t1), slice(None))
                sl1 = (slice(None), slice(t0, t1), slice(0, 1))
                nt_ = t1 - t0
                nc.vector.tensor_reduce(rmax[sl1], ys[sl],
                                        axis=mybir.AxisListType.X, op=ALU.max)
                nc.vector.tensor_tensor(
                    xm[sl], ys[sl],
                    rmax[sl1].to_broadcast([128, nt_, cfg.CLS]),
                    op=ALU.subtract)
                nc.scalar.activation(ex[sl], xm[sl], ACT.Exp)
                nc.vector.tensor_reduce(ssum[sl1], ex[sl],
                                        axis=mybir.AxisListType.X, op=ALU.add)
                nc.scalar.activation(lgs[sl1], ssum[sl1], ACT.Ln)
                nc.vector.tensor_tensor(
                    res[sl], xm[sl],
                    lgs[sl1].to_broadcast([128, nt_, cfg.CLS]),
                    op=ALU.subtract)
                nfull_c = min(t1 * 128, NPC // 128 * 128)
                if nfull_c > c0:
                    nc.sync.dma_start(
                        outd[c0:nfull_c, :].rearrange("(t p) f -> p t f", p=128),
                        res[:, t0:nfull_c // 128, :])
                if t1 * 128 > NPC:
                    tailw = NPC - NPC // 128 * 128
                    nc.sync.dma_start(outd[NPC - tailw:NPC, :],
                                      res[:tailw, NPC // 128, :])

            self.aggregate(4, tfullA[2], tfullB[2], fcols=KC, post_cb=post4)

        nc.compile()
        return nc

    # ---------------- aggregation ----------------
    def aggregate(self, r, tblA, tblB, fcols=128, post_cb=None, mid_cb=None):
        cfg, meta, nc, tc = self.cfg, self.meta, self.nc, self.tc
        NPC, NB = cfg.NPC, cfg.NB
        chunk_list = []
        for h in (0, 1):
            for (t0, ntiles) in meta.chunks[h]:
                chunk_list.append((h, t0, ntiles))
        half_base = [0, meta.half_tiles[0]]

        state = {"ps": None, "b": -1, "h": -1, "next_cb": 0}
        ncb = len(self.node_chunks)

        def emit_ready_cbs(flushed_b):
            # node-chunk k (512 cols) is final once half-1 has flushed its
            # last block min(4k+3, NB-1); emit its dinv[dst] scale + the
            # round's post-aggregation ops inline so they overlap with the
            # remaining aggregation stream.
            while state["next_cb"] < ncb:
                k = state["next_cb"]
                last_b = min(4 * k + 3, NB - 1)
                if flushed_b < last_b:
                    break
                c0, w_ = self.node_chunks[k]
                nc.vector.tensor_tensor(
                    self.acc[:fcols, c0:c0 + w_], self.acc[:fcols, c0:c0 + w_],
                    self.dinvb_t[:fcols, c0:c0 + w_], op=ALU.mult)
                if post_cb is not None:
                    post_cb(k, c0, w_)
                if mid_cb is not None and (k + 1) * 512 == cfg.SPLIT:
                    mid_cb()
                state["next_cb"] += 1

        def flush_group():
            if state["ps"] is None:
                return
            b, h = state["b"], state["h"]
            c0 = b * 128
            w_ = min(NPC - c0, 128)
            if h == 0:
                nc.vector.tensor_copy(self.acc[:fcols, c0:c0 + w_],
                                      state["ps"][:fcols, :w_])
            else:
                nc.vector.tensor_tensor(self.acc[:fcols, c0:c0 + w_],
                                        self.acc[:fcols, c0:c0 + w_],
                                        state["ps"][:fcols, :w_], op=ALU.add)
                emit_ready_cbs(b)
            state["ps"] = None

        def consume(ci, g):
            h, t0, ntiles = chunk_list[ci]
            gt = half_base[h] + t0
            # Build one-hot S on-chip: S = is_equal(iota, dloc)
            s = self.spool.tile([128, cfg.CHUNK_TILES, 128], DT_BF,
                                name="s", tag="s")
            nc.vector.tensor_tensor(
                s[:, :ntiles, :], self.iota_t[:, :ntiles, :],
                self.dloc_t[:, gt:gt + ntiles, :].to_broadcast(
                    [128, ntiles, 128]),
                op=ALU.is_equal)
            for ti in range(ntiles):
                th = t0 + ti
                b = int(meta.tile_block[h][th])
                if b != state["b"] or state["h"] != h:
                    flush_group()
                    state["ps"] = self.pspool.tile([128, 128], DT_F32,
                                                   name="psagg", tag="psagg")
                    state["b"] = b
                    state["h"] = h
                st = (th == meta.block_first[h][b])
                sp = (th == meta.block_last[h][b])
                nc.tensor.matmul(state["ps"][:fcols, :], g[:, ti, :fcols],
                                 s[:, ti, :], start=bool(st), stop=bool(sp))

        n = len(chunk_list)
        for ci in range(n):
            h, t0, ntiles = chunk_list[ci]
            gt = half_base[h] + t0
            g = self.gpool.tile([128, cfg.CHUNK_TILES, 128], DT_BF,
                                name="g", tag="g")
            if r == 1:
                nc.sync.dma_start(g[:, :ntiles, :],
                                  self.g1[:, gt:gt + ntiles, :])
            else:
                q = self.qrr
                self.qrr = (self.qrr + 1) % 4
                n_idx = ntiles * 128
                tbl_ap = (tblA if h == 0 else tblB)[:]
                gi = nc.gpsimd.dma_gather(
                    g[:, :ntiles, :], tbl_ap,
                    self.idx_t[:, gt * 8:(gt + ntiles) * 8],
                    num_idxs=n_idx, num_idxs_reg=n_idx, elem_size=128,
                    single_packet=cfg.SINGLE_PACKET, queue_num=q)
                if self.last_issue[q] is not None:
                    _dep(gi, self.last_issue[q], sync=False)
                self.last_issue[q] = gi
            consume(ci, g)
        flush_group()
        emit_ready_cbs(NB - 1)


def run_hw(inputs_full, cfg: Cfg, trace=False):
    """Full pipeline: host prep, build, run on C cores, unshard."""
    from concourse import bass_utils
    w = {k: np.asarray(inputs_full[k]) for k in
         ('c1_init', 'c1_w', 'c1_root', 'c1_bias',
          'c2_init', 'c2_w', 'c2_root', 'c2_bias')}
    x = np.asarray(inputs_full['x'], dtype=F32)
    ei = np.asarray(inputs_full['edge_index'])
    in_maps, meta = host_prep(x, ei, w, cfg)
    b = Builder(cfg, meta)
    nc = b.build()
    res = bass_utils.run_bass_kernel_spmd(
        nc, in_maps, core_ids=list(range(cfg.C)), trace=trace)
    out = np.concatenate([res.results[c]["out"] for c in range(cfg.C)], axis=0)
    return out, res


_CFG = Cfg(N=50000, E=800000, C=8)


def kernel(**inputs):
    out, _ = run_hw(inputs, _CFG, trace=False)
    return out
